# revision 3
# baseline (speedup 1.0000x reference)
"""Trainium2 Bass kernel for nn_MessageProp (gnn_message_passing).

Reference computation (B=65536 rows, D=128, K=8 components, H=132 hidden):
    msgs  = einsum('kbd,ed->kbe', components, Wm) + bm   # message_map per component
    right = msgs.sum(0) @ Wu.T + bu                      # update_map
    x     = concat([signal, right], -1)
    h0 = relu(x @ W0.T + b0); h1 = relu(h0 @ W1.T + b1); h2 = relu(h1 @ W2.T + b2)
    out = h2 @ W3.T + b3

Key algebraic folds done on the host (all linear maps commute with the k-sum):
    csum = sum_k components[k]
    pre0 = signal @ A.T + csum @ Cm.T + b0'
      A   = W0[:, :D]
      Cm  = W0[:, D:] @ Wu @ Wm
      b0' = b0 + W0[:, D:] @ (Wu @ (K*bm) + bu)
so the device only computes csum (DVE adds over batched loads) and a
4-matmul-layer MLP in feature-major layout.

Memory-bound problem: 40MB HBM traffic per core (32MB components + 4MB signal
in, 4MB out) at ~360GB/s/core aggregate DMA => ~114us floor. Structure keeps
both HWDGE queues 100% on loads:
  - one batched dma_start per 4 components (sync: k=0..3, scalar: k=4..7+sig)
  - output stores issued via Pool SWDGE so they never block load queues
  - component sum: 7 DVE adds (Pool adds are 2.4x slower)
  - MLP feature-major with f32r matmuls (1 cyc/row at N>=512); final layer
    emits row-major output directly via lhsT=h2-block matmuls with bf16
    W3^T moving operand (no output transposes, no extra PSUM bank)

Sharding: data-parallel over B across 8 cores (8192 rows each); weights replicated.
"""

import numpy as np
import ml_dtypes
from contextlib import ExitStack, nullcontext

import concourse.bass as bass
import concourse.bacc as bacc
import concourse.tile as tile
import concourse.mybir as mybir
from concourse import bass_utils

F32 = mybir.dt.float32
R32 = mybir.dt.float32r
BF16 = mybir.dt.bfloat16
ACT = mybir.ActivationFunctionType

D = 128          # latent dim
H = 132          # FCBlock hidden width
B = 65536        # batch
K = 8            # components
NCORES = 8
RB = B // NCORES  # 8192 rows per core
SUB = 4          # m-blocks (128 rows each) per compute sub-tile -> 512 rows

# tapered row-tile sizes (sum = RB); small final tiles shrink the drain tail
TILES = (1024,) * 7 + (512, 512)
BUFS_LOADS = 3
BUFS_ACTS = 3
BUFS_OUT = 2
# PSUM bank budget (8 total)
B_IN = 2
B_HA = 3
B_HB = 2
B_PO = 1
# which engine issues output stores: "gpsimd" keeps HWDGE queues load-only
STORE_ENGINE = "gpsimd"
# engine split for the 7-add merge tree: number done on vector (rest gpsimd)
MERGE_DVE = 7
# repeat whole body via HW loop (timing harness only)
REPS = 1
# timing-only: skip merge+MLP, just do the DMA pattern (output is garbage)
SKIP_COMPUTE = False

# f32 wpack column layout ([128, NW]); see _build_wpack
_C_IDENT = 0
_C_W0A_SIG = 128
_C_W0A_CS = 256
_C_W1A_HI = 384
_C_W2A_HI = 512
_C_W1A_LO = 640    # [4,128] on partitions 0:4
_C_W2A_LO = 768    # [4,128]
_C_W0B_SIG = 896   # [128,4]
_C_W0B_CS = 900
_C_W1B_HI = 904
_C_W2B_HI = 908
_C_W1B_LO = 912    # [4,4]
_C_W2B_LO = 916
_C_B0A = 920
_C_B1A = 921
_C_B2A = 922
_C_B0B = 923       # [4,1]
_C_B1B = 924
_C_B2B = 925
NW = 926

# bf16 wpack column layout ([128, NBF]) for the row-major final layer
_CB_W3T_HI = 0     # [128,128] = W3.T[:128, :]
_CB_W3T_LO = 128   # [4,128]   = W3.T[128:, :]
_CB_B3ROW = 256    # [1,128]   = b3
_CB_ONES = 384     # [1,128]   = 1.0
NBF = 512


def _build_wpack(Wm, bm, Wu, bu, W0, b0, W1, b1, W2, b2, W3, b3):
    f8 = np.float64
    Wm, bm, Wu, bu = Wm.astype(f8), bm.astype(f8), Wu.astype(f8), bu.astype(f8)
    W0, b0, W1, b1 = W0.astype(f8), b0.astype(f8), W1.astype(f8), b1.astype(f8)
    W2, b2 = W2.astype(f8), b2.astype(f8)

    A = W0[:, :D]                              # [H, D]
    W0r = W0[:, D:]                            # [H, D]
    Cm = W0r @ (Wu @ Wm)                       # [H, D]
    b0p = b0 + W0r @ (Wu @ (K * bm) + bu)      # [H]

    w = np.zeros((128, NW), dtype=np.float64)
    w[:, _C_IDENT:_C_IDENT + 128] = np.eye(128)
    # L0: lhsT[p=d, m=h] = A.T / Cm.T
    w[:, _C_W0A_SIG:_C_W0A_SIG + 128] = A.T[:, :128]
    w[:, _C_W0A_CS:_C_W0A_CS + 128] = Cm.T[:, :128]
    w[:, _C_W0B_SIG:_C_W0B_SIG + 4] = A.T[:, 128:]
    w[:, _C_W0B_CS:_C_W0B_CS + 4] = Cm.T[:, 128:]
    # L1/L2: lhsT[p=h_in, m=h_out] = Wx.T
    for Wx, chi, clo, cbhi, cblo in (
        (W1, _C_W1A_HI, _C_W1A_LO, _C_W1B_HI, _C_W1B_LO),
        (W2, _C_W2A_HI, _C_W2A_LO, _C_W2B_HI, _C_W2B_LO),
    ):
        WT = Wx.T                              # [132 in, 132 out]
        w[:, chi:chi + 128] = WT[:128, :128]
        w[:4, clo:clo + 128] = WT[128:, :128]
        w[:, cbhi:cbhi + 4] = WT[:128, 128:]
        w[:4, cblo:cblo + 4] = WT[128:, 128:]
    # biases
    w[:, _C_B0A] = b0p[:128]
    w[:, _C_B1A] = b1[:128]
    w[:, _C_B2A] = b2[:128]
    w[:4, _C_B0B] = b0p[128:]
    w[:4, _C_B1B] = b1[128:]
    w[:4, _C_B2B] = b2[128:]
    return np.ascontiguousarray(w, dtype=np.float32)


def _build_wpack_bf(W3, b3):
    w = np.zeros((128, NBF), dtype=np.float32)
    W3T = W3.astype(np.float64).T              # [132, 128]
    w[:, _CB_W3T_HI:_CB_W3T_HI + 128] = W3T[:128, :]
    w[:4, _CB_W3T_LO:_CB_W3T_LO + 128] = W3T[128:, :]
    w[0, _CB_B3ROW:_CB_B3ROW + 128] = b3.astype(np.float64)
    w[0, _CB_ONES:_CB_ONES + 128] = 1.0
    return np.ascontiguousarray(w.astype(ml_dtypes.bfloat16))


def make_in_maps(signal, components, Wm, bm, Wu, bu, W0, b0, W1, b1, W2, b2,
                 W3, b3):
    """Per-core input dicts from full inputs (host-side shard + weight pack)."""
    signal = np.ascontiguousarray(np.asarray(signal, dtype=np.float32))
    components = np.ascontiguousarray(np.asarray(components, dtype=np.float32))
    ws = [np.asarray(a, dtype=np.float32) for a in
          (Wm, bm, Wu, bu, W0, b0, W1, b1, W2, b2, W3, b3)]
    wpack = _build_wpack(*ws)
    wpack_bf = _build_wpack_bf(ws[10], ws[11])
    in_maps = []
    for c in range(NCORES):
        r0 = c * RB
        in_maps.append({
            "sig": signal[r0:r0 + RB],
            "comp": np.ascontiguousarray(components[:, r0:r0 + RB, :]),
            "wpack": wpack,
            "wpackr": wpack,
            "wpack_bf": wpack_bf,
        })
    return in_maps


def _trace_kernel(nc: bass.Bass):
    assert sum(TILES) == RB and all(tl % (SUB * 128) == 0 for tl in TILES)
    sig = nc.dram_tensor("sig", [RB, D], R32, kind="ExternalInput")
    comp = nc.dram_tensor("comp", [K, RB, D], F32, kind="ExternalInput")
    wpack = nc.dram_tensor("wpack", [128, NW], F32, kind="ExternalInput")
    wpackr = nc.dram_tensor("wpackr", [128, NW], R32, kind="ExternalInput")
    wpack_bf = nc.dram_tensor("wpack_bf", [128, NBF], BF16, kind="ExternalInput")
    out = nc.dram_tensor("out", [RB, D], F32, kind="ExternalOutput")

    # per-tile views; within tile t: row = r0 + p*M_t + m, free layout (m d)
    def tile_views(r0, tl):
        m = tl // 128
        s_v = sig.ap()[r0:r0 + tl, :].rearrange("(p m) d -> p (m d)", p=128, m=m)
        ca_v = comp.ap()[0:4, r0:r0 + tl, :].rearrange(
            "k (p m) d -> p k (m d)", p=128, m=m)
        cb_v = comp.ap()[4:8, r0:r0 + tl, :].rearrange(
            "k (p m) d -> p k (m d)", p=128, m=m)
        o_v = out.ap()[r0:r0 + tl, :].rearrange("(p m) d -> p (m d)", p=128, m=m)
        return s_v, ca_v, cb_v, o_v

    with tile.TileContext(nc) as tc, ExitStack() as ctx:
        wpool = ctx.enter_context(tc.tile_pool(name="weights", bufs=1))
        loads = ctx.enter_context(tc.tile_pool(name="loads", bufs=BUFS_LOADS))
        acts = ctx.enter_context(tc.tile_pool(name="acts", bufs=BUFS_ACTS))
        opool = ctx.enter_context(tc.tile_pool(name="outs", bufs=BUFS_OUT))
        psum = ctx.enter_context(tc.tile_pool(name="psum", bufs=2, space="PSUM"))

        wsb = wpool.tile([128, NW], F32)
        nc.sync.dma_start(wsb[:], wpack.ap())
        wsr = wpool.tile([128, NW], R32)
        nc.sync.dma_start(wsr[:], wpackr.ap())
        wbf = wpool.tile([128, NBF], BF16)
        nc.sync.dma_start(wbf[:], wpack_bf.ap())

        identr = wsr[:, _C_IDENT:_C_IDENT + 128]

        def wcol(c, n=128, parts=128):
            return wsb[:parts, c:c + n]

        def wcolr(c, n=128, parts=128):
            return wsr[:parts, c:c + n]

        store_eng = getattr(nc, STORE_ENGINE)

        with (tc.For_i(0, REPS, 1) if REPS > 1 else nullcontext()):
            r0 = 0
            for t, TLt in enumerate(TILES):
                NSUB = TLt // (SUB * 128)
                sig_v, ca_v, cb_v, out_v = tile_views(r0, TLt)
                r0 += TLt
                sig_nat = loads.tile([128, TLt], R32, tag="sig_nat")
                nc.sync.dma_start(sig_nat[:], sig_v)
                wa = loads.tile([128, 4 * TLt], F32, tag="wa")
                nc.sync.dma_start(wa[:], ca_v)
                wb = loads.tile([128, 4 * TLt], F32, tag="wb")
                nc.scalar.dma_start(wb[:], cb_v)

                out_sb = opool.tile([128, TLt], F32, tag="out_sb")

                if SKIP_COMPUTE:
                    nc.vector.tensor_copy(out_sb[:], sig_nat[:])
                    store_eng.dma_start(out_v, out_sb[:])
                    continue

                # ---- component sum: 7-add merge tree ----
                T = TLt
                cs_sum = loads.tile([128, TLt], R32, tag="cs_sum")
                adds = [
                    (wa[:, 0:T], wa[:, 0:T], wa[:, T:2 * T]),
                    (wb[:, 0:T], wb[:, 0:T], wb[:, T:2 * T]),
                    (wa[:, 2 * T:3 * T], wa[:, 2 * T:3 * T], wa[:, 3 * T:4 * T]),
                    (wb[:, 2 * T:3 * T], wb[:, 2 * T:3 * T], wb[:, 3 * T:4 * T]),
                    (wa[:, 0:T], wa[:, 0:T], wa[:, 2 * T:3 * T]),
                    (wb[:, 0:T], wb[:, 0:T], wb[:, 2 * T:3 * T]),
                    (cs_sum[:], wa[:, 0:T], wb[:, 0:T]),
                ]
                for i, (dst, s0, s1) in enumerate(adds):
                    eng = nc.vector if i < MERGE_DVE else nc.gpsimd
                    eng.tensor_add(dst, s0, s1)

                for s in range(NSUB):
                    cols = slice(s * SUB * 128, (s + 1) * SUB * 128)

                    # ---- transpose signal + csum blocks into feature-major ----
                    ps_sig = psum.tile([128, SUB * 128], R32, tag="ps_in", bufs=B_IN)
                    for j in range(SUB):
                        mb = (s * SUB + j) * 128
                        nc.tensor.transpose(ps_sig[:, j * 128:(j + 1) * 128],
                                            sig_nat[:, mb:mb + 128], identr)
                    sigT = acts.tile([128, SUB * 128], R32, tag="sigT")
                    nc.scalar.activation(sigT[:], ps_sig[:], ACT.Copy)

                    ps_cs = psum.tile([128, SUB * 128], R32, tag="ps_in", bufs=B_IN)
                    for j in range(SUB):
                        mb = (s * SUB + j) * 128
                        nc.tensor.transpose(ps_cs[:, j * 128:(j + 1) * 128],
                                            cs_sum[:, mb:mb + 128], identr)
                    csT = acts.tile([128, SUB * 128], R32, tag="csT")
                    nc.vector.tensor_copy(csT[:], ps_cs[:])

                    # ---- L0: h0 = relu(A@sigT + Cm@csT + b0') ----
                    ps_h0a = psum.tile([128, SUB * 128], F32, tag="ha", bufs=B_HA)
                    nc.tensor.matmul(ps_h0a[:], wcolr(_C_W0A_SIG),
                                     sigT[:], start=True, stop=False)
                    nc.tensor.matmul(ps_h0a[:], wcolr(_C_W0A_CS),
                                     csT[:], start=False, stop=True)
                    ps_h0b = psum.tile([4, SUB * 128], F32, tag="hb", bufs=B_HB)
                    nc.tensor.matmul(ps_h0b[:], wcolr(_C_W0B_SIG, 4),
                                     sigT[:], start=True, stop=False)
                    nc.tensor.matmul(ps_h0b[:], wcolr(_C_W0B_CS, 4),
                                     csT[:], start=False, stop=True)
                    h0a = acts.tile([128, SUB * 128], R32, tag="h0a")
                    nc.vector.tensor_scalar(h0a[:], ps_h0a[:],
                                            wcol(_C_B0A, 1), 0.0,
                                            mybir.AluOpType.add,
                                            mybir.AluOpType.max)
                    h0b = acts.tile([4, SUB * 128], R32, tag="h0b")
                    nc.scalar.activation(h0b[:], ps_h0b[:], ACT.Relu,
                                         bias=wcol(_C_B0B, 1, parts=4))

                    # ---- L1 ----
                    ps_h1a = psum.tile([128, SUB * 128], F32, tag="ha", bufs=B_HA)
                    nc.tensor.matmul(ps_h1a[:], wcolr(_C_W1A_HI),
                                     h0a[:], start=True, stop=False)
                    nc.tensor.matmul(ps_h1a[:], wcolr(_C_W1A_LO, 128, parts=4),
                                     h0b[:], start=False, stop=True)
                    ps_h1b = psum.tile([4, SUB * 128], F32, tag="hb", bufs=B_HB)
                    nc.tensor.matmul(ps_h1b[:], wcolr(_C_W1B_HI, 4),
                                     h0a[:], start=True, stop=False)
                    nc.tensor.matmul(ps_h1b[:], wcolr(_C_W1B_LO, 4, parts=4),
                                     h0b[:], start=False, stop=True)
                    h1a = acts.tile([128, SUB * 128], R32, tag="h1a")
                    nc.vector.tensor_scalar(h1a[:], ps_h1a[:],
                                            wcol(_C_B1A, 1), 0.0,
                                            mybir.AluOpType.add,
                                            mybir.AluOpType.max)
                    h1b = acts.tile([4, SUB * 128], R32, tag="h1b")
                    nc.scalar.activation(h1b[:], ps_h1b[:], ACT.Relu,
                                         bias=wcol(_C_B1B, 1, parts=4))

                    # ---- L2 (outputs cast to bf16 for the L3 stationary) ----
                    ps_h2a = psum.tile([128, SUB * 128], F32, tag="ha", bufs=B_HA)
                    nc.tensor.matmul(ps_h2a[:], wcolr(_C_W2A_HI),
                                     h1a[:], start=True, stop=False)
                    nc.tensor.matmul(ps_h2a[:], wcolr(_C_W2A_LO, 128, parts=4),
                                     h1b[:], start=False, stop=True)
                    ps_h2b = psum.tile([4, SUB * 128], F32, tag="hb", bufs=B_HB)
                    nc.tensor.matmul(ps_h2b[:], wcolr(_C_W2B_HI, 4),
                                     h1a[:], start=True, stop=False)
                    nc.tensor.matmul(ps_h2b[:], wcolr(_C_W2B_LO, 4, parts=4),
                                     h1b[:], start=False, stop=True)
                    h2a = acts.tile([128, SUB * 128], BF16, tag="h2a")
                    nc.scalar.activation(h2a[:], ps_h2a[:], ACT.Relu,
                                         bias=wcol(_C_B2A, 1))
                    h2b = acts.tile([4, SUB * 128], BF16, tag="h2b")
                    nc.scalar.activation(h2b[:], ps_h2b[:], ACT.Relu,
                                         bias=wcol(_C_B2B, 1, parts=4))

                    # ---- L3: row-major out = h2blk.T @ W3T + b3 (bf16) ----
                    ps_o = psum.tile([128, SUB * 128], F32, tag="po", bufs=B_PO)
                    for j in range(SUB):
                        blk = slice(j * 128, (j + 1) * 128)
                        nc.tensor.matmul(ps_o[:, blk], h2a[:, blk],
                                         wbf[:, _CB_W3T_HI:_CB_W3T_HI + 128],
                                         start=True, stop=False)
                        nc.tensor.matmul(ps_o[:, blk], h2b[:4, blk],
                                         wbf[:4, _CB_W3T_LO:_CB_W3T_LO + 128],
                                         start=False, stop=False)
                        nc.tensor.matmul(ps_o[:, blk],
                                         wbf[0:1, _CB_ONES:_CB_ONES + 128],
                                         wbf[0:1, _CB_B3ROW:_CB_B3ROW + 128],
                                         start=False, stop=True)
                    nc.vector.tensor_copy(out_sb[:, cols], ps_o[:])

                store_eng.dma_start(out_v, out_sb[:])

    return nc


_CACHED_NC = None


def _get_nc():
    global _CACHED_NC
    if _CACHED_NC is None:
        nc = bacc.Bacc("TRN2", target_bir_lowering=False, debug=False,
                       enable_asserts=False, num_devices=NCORES)
        _trace_kernel(nc)
        nc.compile()
        _CACHED_NC = nc
    return _CACHED_NC


def kernel(**inputs):
    in_maps = make_in_maps(**inputs)
    nc = _get_nc()
    res = bass_utils.run_bass_kernel_spmd(nc, in_maps,
                                          core_ids=list(range(NCORES)))
    return np.concatenate([res.results[c]["out"] for c in range(NCORES)], axis=0)


# revision 17
# speedup vs baseline: 1.3410x; 1.3410x over previous
"""Trainium2 Bass kernel for nn_MessageProp — all-feature-major variant.

Same algebraic folds as before (see _build_wpack): device computes
    csum = sum_k components[k]          (DVE/Pool adds over DMA'd tiles)
    h0 = relu(A@sigT + Cm@csT + b0'); h1; h2; outT = W3@h2 + b3
entirely in feature-major layout with NO on-device transposes: the HOST
pre-transposes signal and components into per-tile [D, tl] blocks (flat
tensors, one contiguous 512KB block per (tile, k)), and the output is
stored feature-major and transposed back on the host. Host repack time is
not part of device execution time.

This removes all PE transpose matmuls, the sigT/csT psum drains, and the
ps_in psum banks; loads/stores stay perfectly sequential in DRAM.

Sharding: data-parallel over B across 8 cores (8192 rows each).
"""

import numpy as np
from contextlib import ExitStack, nullcontext

import concourse.bass as bass
import concourse.bacc as bacc
import concourse.tile as tile
import concourse.mybir as mybir
from concourse import bass_utils

F32 = mybir.dt.float32
R32 = mybir.dt.float32r
ACT = mybir.ActivationFunctionType

D = 128
H = 132
B = 65536
K = 8
NCORES = 8
RB = B // NCORES
SUB = 4

TILES = (1024,) * 7 + (512, 512)
BUFS_LOADS = 3
BUFS_ACTS = 3
BUFS_OUT = 2
B_HA = 3
B_HB = 2
B_PO = 2
MERGE_DVE = 4
STORE_ENGINE = "gpsimd"
QSPLIT = 4
POOL_LOADS = 0
# engine for h0a/h1a relu drains ("vector") and oT drain ("scalar")
OT_ENG = "scalar"
PASSES = 1
REPS = 1
SKIP_COMPUTE = False

# wpack column layout (fp32, [128, NW])
_C_W0A_SIG = 0
_C_W0A_CS = 128
_C_W1A_HI = 256
_C_W2A_HI = 384
_C_W3_HI = 512
_C_W1A_LO = 640    # [4,128] on partitions 0:4
_C_W2A_LO = 768
_C_W3_LO = 896
_C_W0B_SIG = 1024  # [128,4]
_C_W0B_CS = 1028
_C_W1B_HI = 1032
_C_W2B_HI = 1036
_C_W1B_LO = 1040   # [4,4]
_C_W2B_LO = 1044
_C_B0A = 1048
_C_B1A = 1049
_C_B2A = 1050
_C_B3 = 1051
_C_B0B = 1052      # [4,1]
_C_B1B = 1053
_C_B2B = 1054
NW = 1055


def _build_wpack(Wm, bm, Wu, bu, W0, b0, W1, b1, W2, b2, W3, b3):
    f8 = np.float64
    Wm, bm, Wu, bu = Wm.astype(f8), bm.astype(f8), Wu.astype(f8), bu.astype(f8)
    W0, b0, W1, b1 = W0.astype(f8), b0.astype(f8), W1.astype(f8), b1.astype(f8)
    W2, b2, W3, b3 = W2.astype(f8), b2.astype(f8), W3.astype(f8), b3.astype(f8)

    A = W0[:, :D]
    W0r = W0[:, D:]
    Cm = W0r @ (Wu @ Wm)
    b0p = b0 + W0r @ (Wu @ (K * bm) + bu)

    w = np.zeros((128, NW), dtype=np.float64)
    w[:, _C_W0A_SIG:_C_W0A_SIG + 128] = A.T[:, :128]
    w[:, _C_W0A_CS:_C_W0A_CS + 128] = Cm.T[:, :128]
    w[:, _C_W0B_SIG:_C_W0B_SIG + 4] = A.T[:, 128:]
    w[:, _C_W0B_CS:_C_W0B_CS + 4] = Cm.T[:, 128:]
    for Wx, chi, clo, cbhi, cblo in (
        (W1, _C_W1A_HI, _C_W1A_LO, _C_W1B_HI, _C_W1B_LO),
        (W2, _C_W2A_HI, _C_W2A_LO, _C_W2B_HI, _C_W2B_LO),
    ):
        WT = Wx.T
        w[:, chi:chi + 128] = WT[:128, :128]
        w[:4, clo:clo + 128] = WT[128:, :128]
        w[:, cbhi:cbhi + 4] = WT[:128, 128:]
        w[:4, cblo:cblo + 4] = WT[128:, 128:]
    W3T = W3.T                                 # [132, 128]
    w[:, _C_W3_HI:_C_W3_HI + 128] = W3T[:128, :]
    w[:4, _C_W3_LO:_C_W3_LO + 128] = W3T[128:, :]
    w[:, _C_B0A] = b0p[:128]
    w[:, _C_B1A] = b1[:128]
    w[:, _C_B2A] = b2[:128]
    w[:, _C_B3] = b3
    w[:4, _C_B0B] = b0p[128:]
    w[:4, _C_B1B] = b1[128:]
    w[:4, _C_B2B] = b2[128:]
    return np.ascontiguousarray(w, dtype=np.float32)


def _tile_offsets():
    """(row_start, sig/out flat offset, comp flat offset) per tile, in elems."""
    offs = []
    r0 = 0
    for tl in TILES:
        offs.append((r0, r0 * D, K * r0 * D))
        r0 += tl
    return offs


def make_in_maps(signal, components, Wm, bm, Wu, bu, W0, b0, W1, b1, W2, b2,
                 W3, b3):
    """Per-core input dicts; pre-transposes sig/comp into per-tile [D,tl]
    feature-major blocks (host-side, not device time)."""
    signal = np.asarray(signal, dtype=np.float32)
    components = np.asarray(components, dtype=np.float32)
    wpack = _build_wpack(*[np.asarray(a, dtype=np.float32) for a in
                           (Wm, bm, Wu, bu, W0, b0, W1, b1, W2, b2, W3, b3)])
    in_maps = []
    for c in range(NCORES):
        base = c * RB
        sig_blocks, comp_blocks = [], []
        r0 = 0
        for tl in TILES:
            rows = slice(base + r0, base + r0 + tl)
            sig_blocks.append(np.ascontiguousarray(signal[rows].T).ravel())
            for k in range(K):
                comp_blocks.append(
                    np.ascontiguousarray(components[k, rows].T).ravel())
            r0 += tl
        in_maps.append({
            "sigT": np.concatenate(sig_blocks),
            "compT": np.concatenate(comp_blocks),
            "wpack": wpack,
            "wpackr": wpack,
        })
    return in_maps


def unpack_out(flat):
    """Reassemble row-major [RB, D] from per-tile feature-major blocks."""
    out = np.empty((RB, D), dtype=np.float32)
    r0 = 0
    for tl in TILES:
        blk = flat[r0 * D:(r0 + tl) * D].reshape(D, tl)
        out[r0:r0 + tl] = blk.T
        r0 += tl
    return out


def _trace_kernel(nc: bass.Bass):
    assert sum(TILES) == RB and all(tl % (SUB * 128) == 0 for tl in TILES)
    sigT = nc.dram_tensor("sigT", [RB * D], R32, kind="ExternalInput")
    compT = nc.dram_tensor("compT", [K * RB * D], F32, kind="ExternalInput")
    wpack = nc.dram_tensor("wpack", [128, NW], F32, kind="ExternalInput")
    wpackr = nc.dram_tensor("wpackr", [128, NW], R32, kind="ExternalInput")
    out = nc.dram_tensor("out", [RB * D], F32, kind="ExternalOutput")

    offs = _tile_offsets()

    with tile.TileContext(nc) as tc, ExitStack() as ctx:
        wpool = ctx.enter_context(tc.tile_pool(name="weights", bufs=1))
        loads = ctx.enter_context(tc.tile_pool(name="loads", bufs=BUFS_LOADS))
        acts = ctx.enter_context(tc.tile_pool(name="acts", bufs=BUFS_ACTS))
        opool = ctx.enter_context(tc.tile_pool(name="outs", bufs=BUFS_OUT))
        psum = ctx.enter_context(tc.tile_pool(name="psum", bufs=2, space="PSUM"))

        wsb = wpool.tile([128, NW], F32)
        nc.sync.dma_start(wsb[:], wpack.ap())
        wsr = wpool.tile([128, NW], R32)
        nc.sync.dma_start(wsr[:], wpackr.ap())

        def wcol(c, n=128, parts=128):
            return wsb[:parts, c:c + n]

        def wcolr(c, n=128, parts=128):
            return wsr[:parts, c:c + n]

        store_eng = getattr(nc, STORE_ENGINE)

        with (tc.For_i(0, REPS, 1) if REPS > 1 else nullcontext()):
          for _pass in range(PASSES):
            for t, TLt in enumerate(TILES):
                r0, soff, coff = offs[t]
                sz = TLt * D
                sig_v = sigT.ap()[soff:soff + sz].rearrange(
                    "(d r) -> d r", d=128, r=TLt)
                out_v = out.ap()[soff:soff + sz].rearrange(
                    "(d r) -> d r", d=128, r=TLt)
                ck_v = [compT.ap()[coff + k * sz:coff + (k + 1) * sz].rearrange(
                    "(d r) -> d r", d=128, r=TLt) for k in range(K)]

                sig_nat = loads.tile([128, TLt], R32, tag="sig_nat")
                nc.sync.dma_start(sig_nat[:], sig_v)
                wa = loads.tile([128, 4 * TLt], F32, tag="wa")
                wb = loads.tile([128, 4 * TLt], F32, tag="wb")
                for k in range(K):
                    dst = wa if k < 4 else wb
                    kk = k % 4
                    if k >= K - POOL_LOADS:
                        eng = nc.gpsimd
                    elif k < QSPLIT:
                        eng = nc.sync
                    else:
                        eng = nc.scalar
                    eng.dma_start(dst[:, kk * TLt:(kk + 1) * TLt], ck_v[k])

                out_sb = opool.tile([128, TLt], F32, tag="out_sb")

                if SKIP_COMPUTE:
                    nc.vector.tensor_copy(out_sb[:], sig_nat[:])
                    store_eng.dma_start(out_v, out_sb[:])
                    continue

                # ---- component sum: 7-add merge tree (DVE/Pool split) ----
                T = TLt
                cs_sum = loads.tile([128, TLt], R32, tag="cs_sum")
                adds = [
                    (wa[:, 0:T], wa[:, 0:T], wa[:, T:2 * T]),
                    (wb[:, 0:T], wb[:, 0:T], wb[:, T:2 * T]),
                    (wa[:, 2 * T:3 * T], wa[:, 2 * T:3 * T], wa[:, 3 * T:4 * T]),
                    (wb[:, 2 * T:3 * T], wb[:, 2 * T:3 * T], wb[:, 3 * T:4 * T]),
                    (wa[:, 0:T], wa[:, 0:T], wa[:, 2 * T:3 * T]),
                    (wb[:, 0:T], wb[:, 0:T], wb[:, 2 * T:3 * T]),
                    (cs_sum[:], wa[:, 0:T], wb[:, 0:T]),
                ]
                for i, (dst, s0, s1) in enumerate(adds):
                    eng = nc.vector if i < MERGE_DVE else nc.gpsimd
                    eng.tensor_add(dst, s0, s1)

                for s in range(TLt // (SUB * 128)):
                    cols = slice(s * SUB * 128, (s + 1) * SUB * 128)
                    sigT_s = sig_nat[:, cols]
                    csT_s = cs_sum[:, cols]

                    # ---- L0 ----
                    ps_h0a = psum.tile([128, SUB * 128], F32, tag="ha", bufs=B_HA)
                    nc.tensor.matmul(ps_h0a[:], wcolr(_C_W0A_SIG),
                                     sigT_s, start=True, stop=False)
                    nc.tensor.matmul(ps_h0a[:], wcolr(_C_W0A_CS),
                                     csT_s, start=False, stop=True)
                    ps_h0b = psum.tile([4, SUB * 128], F32, tag="hb", bufs=B_HB)
                    nc.tensor.matmul(ps_h0b[:], wcolr(_C_W0B_SIG, 4),
                                     sigT_s, start=True, stop=False)
                    nc.tensor.matmul(ps_h0b[:], wcolr(_C_W0B_CS, 4),
                                     csT_s, start=False, stop=True)
                    h0a = acts.tile([128, SUB * 128], R32, tag="h0a")
                    nc.vector.tensor_scalar(h0a[:], ps_h0a[:],
                                            wcol(_C_B0A, 1), 0.0,
                                            mybir.AluOpType.add,
                                            mybir.AluOpType.max)
                    h0b = acts.tile([4, SUB * 128], R32, tag="h0b")
                    nc.scalar.activation(h0b[:], ps_h0b[:], ACT.Relu,
                                         bias=wcol(_C_B0B, 1, parts=4))

                    # ---- L1 ----
                    ps_h1a = psum.tile([128, SUB * 128], F32, tag="ha", bufs=B_HA)
                    nc.tensor.matmul(ps_h1a[:], wcolr(_C_W1A_HI),
                                     h0a[:], start=True, stop=False)
                    nc.tensor.matmul(ps_h1a[:], wcolr(_C_W1A_LO, 128, parts=4),
                                     h0b[:], start=False, stop=True)
                    ps_h1b = psum.tile([4, SUB * 128], F32, tag="hb", bufs=B_HB)
                    nc.tensor.matmul(ps_h1b[:], wcolr(_C_W1B_HI, 4),
                                     h0a[:], start=True, stop=False)
                    nc.tensor.matmul(ps_h1b[:], wcolr(_C_W1B_LO, 4, parts=4),
                                     h0b[:], start=False, stop=True)
                    h1a = acts.tile([128, SUB * 128], R32, tag="h1a")
                    nc.vector.tensor_scalar(h1a[:], ps_h1a[:],
                                            wcol(_C_B1A, 1), 0.0,
                                            mybir.AluOpType.add,
                                            mybir.AluOpType.max)
                    h1b = acts.tile([4, SUB * 128], R32, tag="h1b")
                    nc.scalar.activation(h1b[:], ps_h1b[:], ACT.Relu,
                                         bias=wcol(_C_B1B, 1, parts=4))

                    # ---- L2 ----
                    ps_h2a = psum.tile([128, SUB * 128], F32, tag="ha", bufs=B_HA)
                    nc.tensor.matmul(ps_h2a[:], wcolr(_C_W2A_HI),
                                     h1a[:], start=True, stop=False)
                    nc.tensor.matmul(ps_h2a[:], wcolr(_C_W2A_LO, 128, parts=4),
                                     h1b[:], start=False, stop=True)
                    ps_h2b = psum.tile([4, SUB * 128], F32, tag="hb", bufs=B_HB)
                    nc.tensor.matmul(ps_h2b[:], wcolr(_C_W2B_HI, 4),
                                     h1a[:], start=True, stop=False)
                    nc.tensor.matmul(ps_h2b[:], wcolr(_C_W2B_LO, 4, parts=4),
                                     h1b[:], start=False, stop=True)
                    h2a = acts.tile([128, SUB * 128], R32, tag="h2a")
                    nc.scalar.activation(h2a[:], ps_h2a[:], ACT.Relu,
                                         bias=wcol(_C_B2A, 1))
                    h2b = acts.tile([4, SUB * 128], R32, tag="h2b")
                    nc.scalar.activation(h2b[:], ps_h2b[:], ACT.Relu,
                                         bias=wcol(_C_B2B, 1, parts=4))

                    # ---- L3: outT = W3 @ h2 + b3, drained straight to SBUF ----
                    ps_oT = psum.tile([128, SUB * 128], F32, tag="po", bufs=B_PO)
                    nc.tensor.matmul(ps_oT[:], wcolr(_C_W3_HI),
                                     h2a[:], start=True, stop=False)
                    nc.tensor.matmul(ps_oT[:], wcolr(_C_W3_LO, 128, parts=4),
                                     h2b[:], start=False, stop=True)
                    if OT_ENG == "scalar":
                        nc.scalar.activation(out_sb[:, cols], ps_oT[:],
                                             ACT.Identity, bias=wcol(_C_B3, 1))
                    else:
                        nc.vector.tensor_scalar(out_sb[:, cols], ps_oT[:],
                                                wcol(_C_B3, 1), None,
                                                mybir.AluOpType.add)

                store_eng.dma_start(out_v, out_sb[:])

    return nc


_CACHED_NC = None


def _get_nc():
    global _CACHED_NC
    if _CACHED_NC is None:
        nc = bacc.Bacc("TRN2", target_bir_lowering=False, debug=False,
                       enable_asserts=False, num_devices=NCORES)
        _trace_kernel(nc)
        nc.compile()
        _CACHED_NC = nc
    return _CACHED_NC


def kernel(**inputs):
    in_maps = make_in_maps(**inputs)
    nc = _get_nc()
    res = bass_utils.run_bass_kernel_spmd(nc, in_maps,
                                          core_ids=list(range(NCORES)))
    return np.concatenate([unpack_out(res.results[c]["out"])
                           for c in range(NCORES)], axis=0)
